# revision 64
# baseline (speedup 1.0000x reference)
"""Fused attention-encoding kernel for Trainium2, 8-core batch-parallel SPMD.

Problem (per batch b of 16, p=1024 tokens, d=512 features):
    A[i,j] = wa.P_i + wb.P_j + (wc*P_i).P_j        (si = wa.P_i cancels in softmax)
    SA     = softmax_j(A)
    attn   = SA @ P
    Pc     = [P, attn]
    out    = sigmoid(Pc@w2) * P + sigmoid(Pc@w3) * tanh(Pc@w1)

Strategy: batch-parallel over 8 cores (2 batches/core). Per batch, scores are
computed transposed (S^T[j,i], j on partitions) so that
  - sj is precomputed on host and folded into the exp as a per-partition
    activation bias (no rank-1 fold matmuls),
  - the softmax denominator is a ones-matmul over partitions,
  - the attention matmul consumes E=exp(S^T) directly (no transpose of E),
  - attn^T[d,i] lands exactly in the layout the gate matmuls need as lhsT.
Scores / attention / rowsum matmuls run in fp8e4 DoubleRow mode (K=256 per
instruction, 2x bf16 PE rate). The gates run fully fp8: P-halves in e4m3 DR
with 32x-scaled weights (descaled via the activations' scale=1/32), attn-halves
in fp8e5 DR (their contribution to the pre-activations is tiny, so e5m2 noise
is negligible) accumulating into the same PSUM tile. Dual-fp8 LDWEIGHTS/matmul requires the two K-planes of every
DoubleRow operand to be contiguous in SBUF, so all fp8 tiles are laid out
pair-innermost; the host precomputes matching DRAM layouts (no on-chip
transposes at all). Phases are ordered B0,B1,C0,C1,D0,D1 so the ACT engine
switches tables only twice (Exp for both score phases, Sigmoid/Tanh for both
gate phases). The final combine runs in bf16 for 2x DVE throughput; only the
last add upcasts to the fp32 output.
"""

import sys

if "/opt/trn_rl_repo" not in sys.path:
    sys.path.insert(0, "/opt/trn_rl_repo")

from contextlib import ExitStack

import ml_dtypes
import numpy as np

import concourse.bass as bass  # noqa: F401
import concourse.mybir as mybir
import concourse.tile as tile
from concourse import bacc
from concourse.bass_utils import run_bass_kernel_spmd

B, PL, D = 16, 1024, 512
NCORES = 8
BPC = B // NCORES          # batches per core
NI = PL // 128             # token blocks (i or j): 8
ND = D // 128              # feature chunks: 4
NDP = ND // 2              # feature chunk pairs (DoubleRow): 2
NJP = NI // 2              # token chunk pairs (DoubleRow): 4
WC_SCALE = 32.0            # pw8 = P^T * (WC_SCALE*wc), descaled in the exp
FP32 = mybir.dt.float32
BF16 = mybir.dt.bfloat16
FP8E4 = mybir.dt.float8e4
FP8E5 = mybir.dt.float8e5
AF = mybir.ActivationFunctionType
DR = mybir.MatmulPerfMode.DoubleRow

_cache = {}


def _build(with_bias: bool, taps: tuple = ()):
    nc = bacc.Bacc(
        "TRN2", target_bir_lowering=False, debug=False, num_devices=1
    )
    # pt8: [dp, jb, cp, t, m]   lhsT of scores   (d=(cp,t,dp) planes pair-contig)
    pt8_d = nc.dram_tensor("pt8", [BPC, 128, NI, NDP, 2, 128], FP8E4, kind="ExternalInput").ap()
    # pw8: [dp, cp, ih, t, ii]  rhs of scores
    pw8_d = nc.dram_tensor("pw8", [BPC, 128, NDP, 2, 2, 512], FP8E4, kind="ExternalInput").ap()
    # pn8: [jp_, dc, jp, t, dcol] lhsT of attn   (j=(jp,t,jp_) planes pair-contig)
    pn8_d = nc.dram_tensor("pn8", [BPC, 128, ND, NJP, 2, 128], FP8E4, kind="ExternalInput").ap()
    p16_d = nc.dram_tensor("p16", [BPC, PL, D], BF16, kind="ExternalInput").ap()
    sj_d = nc.dram_tensor("sj", [BPC, NI, 128], FP32, kind="ExternalInput").ap()
    w8f_d = nc.dram_tensor("w8f", [3, ND, 128, D], FP8E4, kind="ExternalInput").ap()
    w8_d = nc.dram_tensor("w8", [3, 128, ND, D], FP8E5, kind="ExternalInput").ap()
    if with_bias:
        b_d = nc.dram_tensor("b32", [3, D], FP32, kind="ExternalInput").ap()
    out_d = nc.dram_tensor("out", [BPC, PL, D], FP32, kind="ExternalOutput").ap()
    tap_d = {}

    with tile.TileContext(nc) as tc, ExitStack() as ctx:
        pool = lambda name, bufs: ctx.enter_context(
            tc.tile_pool(name=name, bufs=bufs)
        )
        const = pool("const", 1)
        wpool = pool("wts", 1)
        pt8p = pool("pt8", 2)
        pw8p = pool("pw8", 2)
        pn8p = pool("pn8", 2)
        pn16p = pool("pn16", 2)
        sjp = pool("sj", 2)
        e8p = pool("e8", 2)
        at8p = pool("at8", 2)
        rbp = pool("rb", 2)
        gp = pool("gates", 2)
        tmpp = pool("tmp", 2)
        op = pool("outs", 3)
        psmm = ctx.enter_context(tc.tile_pool(name="psmm", bufs=6, space="PSUM"))
        psvec = ctx.enter_context(tc.tile_pool(name="psvec", bufs=2, space="PSUM"))

        def tap(name, ap, lb=0):
            if lb != 0 or name not in taps:
                return
            t = nc.dram_tensor(
                f"tap_{name}", list(ap.shape), ap.dtype, kind="ExternalOutput"
            ).ap()
            tap_d[name] = t
            nc.sync.dma_start(t, ap)

        # --- constants / weights (once) ---
        w8_sb = [wpool.tile([128, ND, D], FP8E5, tag=f"w8_{g}", name=f"w8_{g}") for g in range(3)]
        # P-half weights, 32x-scaled e4m3 (all three psums are 32x; the
        # activations descale via scale=1/32).
        w8f_sb = [wpool.tile([128, ND, D], FP8E4, tag=f"w8f{g}", name=f"w8f{g}") for g in range(3)]

        def load_weights():
            # sync-ring position = HBM priority: issued after the scores-path
            # and attn loads, before the combine operands
            for g in range(3):
                nc.sync.dma_start(
                    w8f_sb[g][:], w8f_d[g].rearrange("c p d -> p c d")
                )
            for g in range(3):
                nc.sync.dma_start(w8_sb[g][:], w8_d[g])

        # M=128 ones: rowsum lands replicated on all 128 PSUM partitions, so the
        # reciprocal writes the broadcast tile directly (no gpsimd broadcast).
        ones8 = const.tile([128, 2, 128], FP8E4, tag="ones8")
        nc.vector.memset(ones8[:], 1.0)
        # PE p-state warmup: dummy DR matmuls on memset data during the initial
        # DMA wait window, so the first real matmuls run at full clock. Sized
        # to finish before the first score operands land (~11us).
        warm_rhs = const.tile([128, 2, 512], FP8E4, tag="warm_rhs")
        nc.vector.memset(warm_rhs[:], 0.0)
        ps_warm = psvec.tile([128, 512], FP32, tag="psvec", name="ps_warm")
        for wi in range(13):
            nc.tensor.matmul(
                ps_warm[:],
                ones8[:],
                warm_rhs[:],
                start=(wi == 0),
                stop=(wi == 12),
                perf_mode=DR,
            )
        if with_bias:
            bb = [const.tile([128, D], FP32, tag=f"bias{g}", name=f"bias{g}") for g in range(3)]
            btmp = const.tile([1, 3 * D], FP32, tag="btmp")
            nc.sync.dma_start(btmp[:], b_d.rearrange("g e -> (g e)")[None, :])
            for g in range(3):
                nc.gpsimd.partition_broadcast(
                    bb[g][:], btmp[0:1, g * D : (g + 1) * D]
                )

        # ---------- loads, both batches (sync-ring order = HBM priority) ----------
        # scores b0 -> scores b1 -> attn (pn8) -> gate weights -> combine
        sj_sb, pt8_, pw8_, pn8_, pn16_ = [], [], [], [], []
        for lb in range(BPC):
            sj = sjp.tile([128, NI], FP32, tag="sj", name=f"sj{lb}")
            nc.sync.dma_start(sj[:], sj_d[lb].rearrange("c p -> p c"))
            sj_sb.append(sj)
            pt8 = pt8p.tile([128, NI, NDP, 2, 128], FP8E4, tag="pt8", name=f"pt8{lb}")
            pw8 = pw8p.tile([128, NDP, 2, 2, 512], FP8E4, tag="pw8", name=f"pw8{lb}")
            if lb == 0:
                # fine-grained first chunks on the sync ring so the first
                # matmul's operands land on 5 parallel DMA queues (~64KB each)
                nc.sync.dma_start(pt8[:, 0:1], pt8_d[lb, :, 0:1])
                for t in range(2):
                    for c0 in (0, 256):
                        nc.sync.dma_start(
                            pw8[:, 0, 0, t, c0 : c0 + 256],
                            pw8_d[lb, :, 0, 0, t, c0 : c0 + 256],
                        )
                nc.sync.dma_start(pw8[:, 0, 1], pw8_d[lb, :, 0, 1])
                nc.sync.dma_start(pw8[:, 1], pw8_d[lb, :, 1])
                for jb in range(1, 4):
                    nc.sync.dma_start(pt8[:, jb : jb + 1], pt8_d[lb, :, jb : jb + 1])
                nc.sync.dma_start(pt8[:, 4:6], pt8_d[lb, :, 4:6])
                nc.sync.dma_start(pt8[:, 6:8], pt8_d[lb, :, 6:8])
            else:
                for cp in range(NDP):
                    nc.sync.dma_start(pw8[:, cp], pw8_d[lb, :, cp])
                for jh in range(2):
                    nc.sync.dma_start(
                        pt8[:, jh * 4 : (jh + 1) * 4], pt8_d[lb, :, jh * 4 : (jh + 1) * 4]
                    )
            pt8_.append(pt8)
            pw8_.append(pw8)
        for lb in range(BPC):
            pn8 = pn8p.tile([128, ND, NJP, 2, 128], FP8E4, tag="pn8", name=f"pn8{lb}")
            for dh in range(2):
                nc.sync.dma_start(
                    pn8[:, dh * 2 : (dh + 1) * 2], pn8_d[lb, :, dh * 2 : (dh + 1) * 2]
                )
            pn8_.append(pn8)
        load_weights()
        for lb in range(BPC):
            pn16 = pn16p.tile([128, NI, D], BF16, tag="pn16", name=f"pn16{lb}")
            nc.sync.dma_start(
                pn16[:], p16_d[lb].rearrange("(i p) d -> p i d", p=128)
            )
            pn16_.append(pn16)
        tap("pt8", pt8_[0][:])
        tap("pw8", pw8_[0][:])
        tap("sj", sj_sb[0][:])

        # ---------- phase B x2: scores + exp + rowsum (fp8 DoubleRow) ----------
        e8_, rb_ = [], []
        for lb in range(BPC):
            # e8: [jp_, jp, ih, t, ii] so attn-rhs / rowsum-rhs pairs are contiguous
            e8 = e8p.tile([128, NJP, 2, 2, 512], FP8E4, tag="e8", name=f"e8{lb}")
            e8_.append(e8)
            for jb in range(NI):
                jp, t = jb // 2, jb % 2
                ps_s = [psmm.tile([128, 512], FP32, tag="psmm", name=f"pss{lb}_{jb}_{_}") for _ in range(2)]
                for cp in range(NDP):
                    lhsT = pt8_[lb][:, jb, cp]
                    for ih in range(2):
                        nc.tensor.matmul(
                            ps_s[ih],
                            lhsT,
                            pw8_[lb][:, cp, ih],
                            start=(cp == 0),
                            stop=(cp == NDP - 1),
                            perf_mode=DR,
                        )
                for ih in range(2):
                    nc.scalar.activation(
                        e8[:, jp, ih, t, :],
                        ps_s[ih][:],
                        AF.Exp,
                        bias=sj_sb[lb][:, jb : jb + 1],
                        scale=1.0 / WC_SCALE,
                    )
            # rowsum at the end: uninterrupted DR stream, ones[128,2,128] -> [128,512]
            ps_rs = [psvec.tile([128, 512], FP32, tag="psvec", name=f"psrs{lb}_{_}") for _ in range(2)]
            for jp in range(NJP):
                for ih in range(2):
                    nc.tensor.matmul(
                        ps_rs[ih][:],
                        ones8[:],
                        e8[:, jp, ih],
                        start=(jp == 0),
                        stop=(jp == NJP - 1),
                        perf_mode=DR,
                    )
            rb32 = rbp.tile([128, 2, 512], FP32, tag="rb32", name=f"rb32{lb}")
            for ih in range(2):
                nc.vector.reciprocal_approx_fast(out=rb32[:, ih], in_=ps_rs[ih][:])
            rb_.append(rb32)
        tap("e8", e8_[0][:])
        tap("rb32", rb_[0][:])

        # ---------- phase C x2: attn^T + normalize (fp8 DoubleRow) ----------
        at8_ = []
        for lb in range(BPC):
            # at8: [dp, cp, ib, t, m] so gate lhsT pairs are contiguous
            at8 = at8p.tile([128, NDP, NI, 2, 128], FP8E5, tag="at8", name=f"at8{lb}")
            at8_.append(at8)
            for dc in range(ND):
                cp, t = dc // 2, dc % 2
                ps_a = [psmm.tile([128, 512], FP32, tag="psmm", name=f"psa{lb}_{dc}_{_}") for _ in range(2)]
                for jp in range(NJP):
                    lhsT = pn8_[lb][:, dc, jp]
                    for ih in range(2):
                        nc.tensor.matmul(
                            ps_a[ih],
                            lhsT,
                            e8_[lb][:, jp, ih],
                            start=(jp == 0),
                            stop=(jp == NJP - 1),
                            perf_mode=DR,
                        )
                for ih in range(2):
                    nc.vector.tensor_mul(
                        at8[:, cp, ih * 4 : (ih + 1) * 4, t, :],
                        ps_a[ih].rearrange("p (i m) -> p i m", m=128),
                        rb_[lb][:, ih].rearrange("p (i m) -> p i m", m=128),
                    )
        tap("at8", at8_[0][:])

        # ---------- phase D x2: gates + combine ----------
        for lb in range(BPC):
            for ib in range(NI):
                ps_g = [psmm.tile([128, 512], FP32, tag="psmm", name=f"psg{lb}_{ib}_{_}") for _ in range(3)]
                # all gates fully fp8: P-half e4m3 DR reusing pt8 as lhsT
                # (32x-scaled weights; activations descale via scale=1/32)
                for cp in range(NDP):
                    for g in range(3):
                        nc.tensor.matmul(
                            ps_g[g],
                            pt8_[lb][:, ib, cp],
                            w8f_sb[g][:, 2 * cp : 2 * cp + 2, :],
                            start=(cp == 0),
                            stop=False,
                            perf_mode=DR,
                        )
                for cp in range(NDP):
                    lhsT8 = at8_[lb][:, cp, ib]
                    for g in range(3):
                        nc.tensor.matmul(
                            ps_g[g],
                            lhsT8,
                            w8_sb[g][:, 2 * cp : 2 * cp + 2, :],
                            start=False,
                            stop=(cp == NDP - 1),
                            perf_mode=DR,
                        )
                if with_bias:
                    # bb[1]/bb[2] are pre-scaled by 32 on the host (32x psums)
                    for g in range(3):
                        nc.vector.tensor_add(ps_g[g][:], ps_g[g][:], bb[g][:])
                z16 = gp.tile([128, D], BF16, tag="z16")
                r16 = gp.tile([128, D], BF16, tag="r16")
                f16 = gp.tile([128, D], BF16, tag="f16")
                t16 = tmpp.tile([128, D], BF16, tag="t16")
                rp16 = tmpp.tile([128, D], BF16, tag="rp16")
                o32 = op.tile([128, D], FP32, tag="o32")
                # final block: pipeline the combine in two free-dim halves so
                # the first store overlaps the second half's activations
                # (engines are 128-lane parallel: only free-dim splits shorten
                # op latency)
                halves = (
                    [(0, 256), (256, 512)]
                    if (lb == BPC - 1 and ib == NI - 1)
                    else [(0, 512)]
                )
                for c0, c1 in halves:
                    nc.scalar.activation(
                        z16[:, c0:c1], ps_g[0][:, c0:c1], AF.Tanh, scale=1.0 / 32.0
                    )
                    nc.scalar.activation(
                        f16[:, c0:c1], ps_g[2][:, c0:c1], AF.Sigmoid, scale=1.0 / 32.0
                    )
                    nc.vector.tensor_mul(t16[:, c0:c1], f16[:, c0:c1], z16[:, c0:c1])
                    nc.scalar.activation(
                        r16[:, c0:c1], ps_g[1][:, c0:c1], AF.Sigmoid, scale=1.0 / 32.0
                    )
                    nc.vector.tensor_mul(
                        rp16[:, c0:c1], r16[:, c0:c1], pn16_[lb][:, ib, c0:c1]
                    )
                    nc.vector.tensor_add(o32[:, c0:c1], rp16[:, c0:c1], t16[:, c0:c1])
                    if lb == BPC - 1 and ib == NI - 1:
                        # tail: 64KB store pieces drain on parallel DMA queues
                        cm = (c0 + c1) // 2
                        nc.sync.dma_start(
                            out_d[lb, ib * 128 : (ib + 1) * 128, c0:cm], o32[:, c0:cm]
                        )
                        nc.sync.dma_start(
                            out_d[lb, ib * 128 : (ib + 1) * 128, cm:c1], o32[:, cm:c1]
                        )
                    else:
                        nc.sync.dma_start(
                            out_d[lb, ib * 128 : (ib + 1) * 128, c0:c1], o32[:, c0:c1]
                        )

    nc.compile()
    return nc


def _get_nc(with_bias: bool):
    if with_bias not in _cache:
        _cache[with_bias] = _build(with_bias)
    return _cache[with_bias]


def _prep_in_maps(P, w_atten, w1, w2, w3, b1, b2, b3):
    P = np.ascontiguousarray(np.asarray(P, dtype=np.float32))
    w_atten = np.asarray(w_atten, dtype=np.float32)
    wb = w_atten[D : 2 * D]
    wc = w_atten[2 * D :]
    sj = (P @ wb).reshape(B, NI, 128).astype(np.float32)

    P8 = P.astype(ml_dtypes.float8_e4m3)                       # (B, PL, D)
    Pw8 = (P * (WC_SCALE * wc)[None, None, :]).astype(ml_dtypes.float8_e4m3)
    # pt8: [b, dp, jb, cp, t, m] from P^T[d=(cp,t,dp), j=(jb,m)]
    pt8 = np.ascontiguousarray(
        P8.transpose(0, 2, 1).reshape(B, NDP, 2, 128, NI, 128).transpose(0, 3, 4, 1, 2, 5)
    )
    # pw8: [b, dp, cp, ih, t, ii] from Pw^T[d=(cp,t,dp), i=(ih,ii)]
    pw8 = np.ascontiguousarray(
        Pw8.transpose(0, 2, 1).reshape(B, NDP, 2, 128, 2, 512).transpose(0, 3, 1, 4, 2, 5)
    )
    # pn8: [b, jp_, dc, jp, t, dcol] from P[j=(jp,t,jp_), d=(dc,dcol)]
    pn8 = np.ascontiguousarray(
        P8.reshape(B, NJP, 2, 128, ND, 128).transpose(0, 3, 4, 1, 2, 5)
    )
    P16 = P.astype(ml_dtypes.bfloat16)

    w_all = np.stack(
        [np.asarray(w, dtype=np.float32) for w in (w1, w2, w3)]
    )                                                           # (3, 2D, D)
    w8f = np.ascontiguousarray(
        (32.0 * w_all[:, :D]).reshape(3, ND, 128, D)
    ).astype(ml_dtypes.float8_e4m3)                             # (3, ND, 128, D)
    w8 = np.ascontiguousarray(
        (32.0 * w_all[:, D:]).reshape(3, ND, 128, D).transpose(0, 2, 1, 3)
    ).astype(ml_dtypes.float8_e5m2)                             # (3, 128, ND, D)

    biases = np.stack([np.asarray(b, dtype=np.float32) for b in (b1, b2, b3)])
    biases *= 32.0                                              # all psums are 32x
    with_bias = bool(np.any(biases))
    base = {"w8": w8, "w8f": w8f}
    if with_bias:
        base["b32"] = biases
    in_maps = []
    for c in range(NCORES):
        s = slice(c * BPC, (c + 1) * BPC)
        m = dict(base)
        m["p16"] = P16[s]
        m["pt8"] = pt8[s]
        m["pw8"] = pw8[s]
        m["pn8"] = pn8[s]
        m["sj"] = sj[s]
        in_maps.append(m)
    return in_maps, with_bias


def run(P, w_atten, w1, w2, w3, b1, b2, b3, trace=False):
    in_maps, with_bias = _prep_in_maps(P, w_atten, w1, w2, w3, b1, b2, b3)
    nc = _get_nc(with_bias)
    res = run_bass_kernel_spmd(
        nc, in_maps, core_ids=list(range(NCORES)), trace=trace
    )
    out = np.concatenate([res.results[c]["out"] for c in range(NCORES)], axis=0)
    return out, res


def kernel(P, w_atten, w1, w2, w3, b1, b2, b3):
    out, _ = run(P, w_atten, w1, w2, w3, b1, b2, b3)
    return out


# revision 67
# speedup vs baseline: 1.0081x; 1.0081x over previous
"""Fused attention-encoding kernel for Trainium2, 8-core batch-parallel SPMD.

Problem (per batch b of 16, p=1024 tokens, d=512 features):
    A[i,j] = wa.P_i + wb.P_j + (wc*P_i).P_j        (si = wa.P_i cancels in softmax)
    SA     = softmax_j(A)
    attn   = SA @ P
    Pc     = [P, attn]
    out    = sigmoid(Pc@w2) * P + sigmoid(Pc@w3) * tanh(Pc@w1)

Strategy: batch-parallel over 8 cores (2 batches/core). Per batch, scores are
computed transposed (S^T[j,i], j on partitions) so that
  - sj is precomputed on host and folded into the exp as a per-partition
    activation bias (no rank-1 fold matmuls),
  - the softmax denominator is a ones-matmul over partitions,
  - the attention matmul consumes E=exp(S^T) directly (no transpose of E),
  - attn^T[d,i] lands exactly in the layout the gate matmuls need as lhsT.
Scores / attention / rowsum matmuls run in fp8e4 DoubleRow mode (K=256 per
instruction, 2x bf16 PE rate). The gates run fully fp8: P-halves in e4m3 DR
with 32x-scaled weights (descaled via the activations' scale=1/32), attn-halves
in fp8e5 DR (their contribution to the pre-activations is tiny, so e5m2 noise
is negligible) accumulating into the same PSUM tile. Dual-fp8 LDWEIGHTS/matmul requires the two K-planes of every
DoubleRow operand to be contiguous in SBUF, so all fp8 tiles are laid out
pair-innermost; the host precomputes matching DRAM layouts (no on-chip
transposes at all). Phases are ordered B0,B1,C0,C1,D0,D1 so the ACT engine
switches tables only twice (Exp for both score phases, Sigmoid/Tanh for both
gate phases). The final combine runs in bf16 for 2x DVE throughput; only the
last add upcasts to the fp32 output.
"""

import sys

if "/opt/trn_rl_repo" not in sys.path:
    sys.path.insert(0, "/opt/trn_rl_repo")

from contextlib import ExitStack

import ml_dtypes
import numpy as np

import concourse.bass as bass  # noqa: F401
import concourse.mybir as mybir
import concourse.tile as tile
from concourse import bacc
from concourse.bass_utils import run_bass_kernel_spmd

B, PL, D = 16, 1024, 512
NCORES = 8
BPC = B // NCORES          # batches per core
NI = PL // 128             # token blocks (i or j): 8
ND = D // 128              # feature chunks: 4
NDP = ND // 2              # feature chunk pairs (DoubleRow): 2
NJP = NI // 2              # token chunk pairs (DoubleRow): 4
WC_SCALE = 32.0            # pw8 = P^T * (WC_SCALE*wc), descaled in the exp
FP32 = mybir.dt.float32
BF16 = mybir.dt.bfloat16
FP8E4 = mybir.dt.float8e4
FP8E5 = mybir.dt.float8e5
AF = mybir.ActivationFunctionType
DR = mybir.MatmulPerfMode.DoubleRow

_cache = {}


def _build(with_bias: bool, taps: tuple = ()):
    nc = bacc.Bacc(
        "TRN2", target_bir_lowering=False, debug=False, num_devices=1
    )
    # pt8: [dp, jb, cp, t, m]   lhsT of scores   (d=(cp,t,dp) planes pair-contig)
    pt8_d = nc.dram_tensor("pt8", [BPC, 128, NI, NDP, 2, 128], FP8E4, kind="ExternalInput").ap()
    # pw8: [dp, cp, ih, t, ii]  rhs of scores
    pw8_d = nc.dram_tensor("pw8", [BPC, 128, NDP, 2, 2, 512], FP8E4, kind="ExternalInput").ap()
    # pn8: [jp_, dc, jp, t, dcol] lhsT of attn   (j=(jp,t,jp_) planes pair-contig)
    pn8_d = nc.dram_tensor("pn8", [BPC, 128, ND, NJP, 2, 128], FP8E4, kind="ExternalInput").ap()
    p16_d = nc.dram_tensor("p16", [BPC, PL, D], BF16, kind="ExternalInput").ap()
    sj_d = nc.dram_tensor("sj", [BPC, NI, 128], FP32, kind="ExternalInput").ap()
    w8f_d = nc.dram_tensor("w8f", [3, ND, 128, D], FP8E4, kind="ExternalInput").ap()
    w8_d = nc.dram_tensor("w8", [3, 128, ND, D], FP8E5, kind="ExternalInput").ap()
    if with_bias:
        b_d = nc.dram_tensor("b32", [3, D], FP32, kind="ExternalInput").ap()
    out_d = nc.dram_tensor("out", [BPC, PL, D], FP32, kind="ExternalOutput").ap()
    tap_d = {}

    with tile.TileContext(nc) as tc, ExitStack() as ctx:
        pool = lambda name, bufs: ctx.enter_context(
            tc.tile_pool(name=name, bufs=bufs)
        )
        const = pool("const", 1)
        wpool = pool("wts", 1)
        pt8p = pool("pt8", 2)
        pw8p = pool("pw8", 2)
        pn8p = pool("pn8", 2)
        pn16p = pool("pn16", 2)
        sjp = pool("sj", 2)
        e8p = pool("e8", 2)
        at8p = pool("at8", 2)
        rbp = pool("rb", 2)
        gp = pool("gates", 2)
        tmpp = pool("tmp", 2)
        op = pool("outs", 3)
        psmm = ctx.enter_context(tc.tile_pool(name="psmm", bufs=6, space="PSUM"))
        psvec = ctx.enter_context(tc.tile_pool(name="psvec", bufs=2, space="PSUM"))

        def tap(name, ap, lb=0):
            if lb != 0 or name not in taps:
                return
            t = nc.dram_tensor(
                f"tap_{name}", list(ap.shape), ap.dtype, kind="ExternalOutput"
            ).ap()
            tap_d[name] = t
            nc.sync.dma_start(t, ap)

        # --- constants / weights (once) ---
        w8_sb = [wpool.tile([128, ND, D], FP8E5, tag=f"w8_{g}", name=f"w8_{g}") for g in range(3)]
        # P-half weights, 32x-scaled e4m3 (all three psums are 32x; the
        # activations descale via scale=1/32).
        w8f_sb = [wpool.tile([128, ND, D], FP8E4, tag=f"w8f{g}", name=f"w8f{g}") for g in range(3)]

        def load_weights():
            # sync-ring position = HBM priority: issued after the scores-path
            # and attn loads, before the combine operands
            for g in range(3):
                nc.sync.dma_start(
                    w8f_sb[g][:], w8f_d[g].rearrange("c p d -> p c d")
                )
            for g in range(3):
                nc.sync.dma_start(w8_sb[g][:], w8_d[g])

        # M=128 ones: rowsum lands replicated on all 128 PSUM partitions, so the
        # reciprocal writes the broadcast tile directly (no gpsimd broadcast).
        ones8 = const.tile([128, 2, 128], FP8E4, tag="ones8")
        nc.vector.memset(ones8[:], 1.0)
        # PE p-state warmup: dummy DR matmuls on memset data during the initial
        # DMA wait window, so the first real matmuls run at full clock. Sized
        # to finish before the first score operands land (~11us).
        warm_rhs = const.tile([128, 2, 512], FP8E4, tag="warm_rhs")
        nc.vector.memset(warm_rhs[:], 0.0)
        ps_warm = psvec.tile([128, 512], FP32, tag="psvec", name="ps_warm")
        for wi in range(13):
            nc.tensor.matmul(
                ps_warm[:],
                ones8[:],
                warm_rhs[:],
                start=(wi == 0),
                stop=(wi == 12),
                perf_mode=DR,
            )
        if with_bias:
            bb = [const.tile([128, D], FP32, tag=f"bias{g}", name=f"bias{g}") for g in range(3)]
            btmp = const.tile([1, 3 * D], FP32, tag="btmp")
            nc.sync.dma_start(btmp[:], b_d.rearrange("g e -> (g e)")[None, :])
            for g in range(3):
                nc.gpsimd.partition_broadcast(
                    bb[g][:], btmp[0:1, g * D : (g + 1) * D]
                )

        # ---------- loads, both batches (sync-ring order = HBM priority) ----------
        # scores b0 -> scores b1 -> attn (pn8) -> gate weights -> combine
        sj_sb, pt8_, pw8_, pn8_, pn16_ = [], [], [], [], []
        for lb in range(BPC):
            sj = sjp.tile([128, NI], FP32, tag="sj", name=f"sj{lb}")
            nc.sync.dma_start(sj[:], sj_d[lb].rearrange("c p -> p c"))
            sj_sb.append(sj)
            pt8 = pt8p.tile([128, NI, NDP, 2, 128], FP8E4, tag="pt8", name=f"pt8{lb}")
            pw8 = pw8p.tile([128, NDP, 2, 2, 512], FP8E4, tag="pw8", name=f"pw8{lb}")
            if lb == 0:
                # fine-grained first chunks on the sync ring so the first
                # matmul's operands land on 5 parallel DMA queues (~64KB each)
                nc.sync.dma_start(pt8[:, 0:1], pt8_d[lb, :, 0:1])
                for t in range(2):
                    for c0 in (0, 256):
                        nc.sync.dma_start(
                            pw8[:, 0, 0, t, c0 : c0 + 256],
                            pw8_d[lb, :, 0, 0, t, c0 : c0 + 256],
                        )
                nc.sync.dma_start(pw8[:, 0, 1], pw8_d[lb, :, 0, 1])
                nc.sync.dma_start(pw8[:, 1], pw8_d[lb, :, 1])
                nc.sync.dma_start(pt8[:, 1:4], pt8_d[lb, :, 1:4])
                nc.sync.dma_start(pt8[:, 4:8], pt8_d[lb, :, 4:8])
            else:
                for cp in range(NDP):
                    nc.sync.dma_start(pw8[:, cp], pw8_d[lb, :, cp])
                for jh in range(2):
                    nc.sync.dma_start(
                        pt8[:, jh * 4 : (jh + 1) * 4], pt8_d[lb, :, jh * 4 : (jh + 1) * 4]
                    )
            pt8_.append(pt8)
            pw8_.append(pw8)
        for lb in range(BPC):
            pn8 = pn8p.tile([128, ND, NJP, 2, 128], FP8E4, tag="pn8", name=f"pn8{lb}")
            for dh in range(2):
                nc.sync.dma_start(
                    pn8[:, dh * 2 : (dh + 1) * 2], pn8_d[lb, :, dh * 2 : (dh + 1) * 2]
                )
            pn8_.append(pn8)
        load_weights()
        for lb in range(BPC):
            pn16 = pn16p.tile([128, NI, D], BF16, tag="pn16", name=f"pn16{lb}")
            nc.sync.dma_start(
                pn16[:], p16_d[lb].rearrange("(i p) d -> p i d", p=128)
            )
            pn16_.append(pn16)
        tap("pt8", pt8_[0][:])
        tap("pw8", pw8_[0][:])
        tap("sj", sj_sb[0][:])

        # ---------- phase B x2: scores + exp + rowsum (fp8 DoubleRow) ----------
        e8_, rb_ = [], []
        for lb in range(BPC):
            # e8: [jp_, jp, ih, t, ii] so attn-rhs / rowsum-rhs pairs are contiguous
            e8 = e8p.tile([128, NJP, 2, 2, 512], FP8E4, tag="e8", name=f"e8{lb}")
            e8_.append(e8)
            for jb in range(NI):
                jp, t = jb // 2, jb % 2
                ps_s = [psmm.tile([128, 512], FP32, tag="psmm", name=f"pss{lb}_{jb}_{_}") for _ in range(2)]
                for cp in range(NDP):
                    lhsT = pt8_[lb][:, jb, cp]
                    for ih in range(2):
                        nc.tensor.matmul(
                            ps_s[ih],
                            lhsT,
                            pw8_[lb][:, cp, ih],
                            start=(cp == 0),
                            stop=(cp == NDP - 1),
                            perf_mode=DR,
                        )
                for ih in range(2):
                    nc.scalar.activation(
                        e8[:, jp, ih, t, :],
                        ps_s[ih][:],
                        AF.Exp,
                        bias=sj_sb[lb][:, jb : jb + 1],
                        scale=1.0 / WC_SCALE,
                    )
            # rowsum at the end: uninterrupted DR stream, ones[128,2,128] -> [128,512]
            ps_rs = [psvec.tile([128, 512], FP32, tag="psvec", name=f"psrs{lb}_{_}") for _ in range(2)]
            for jp in range(NJP):
                for ih in range(2):
                    nc.tensor.matmul(
                        ps_rs[ih][:],
                        ones8[:],
                        e8[:, jp, ih],
                        start=(jp == 0),
                        stop=(jp == NJP - 1),
                        perf_mode=DR,
                    )
            rb32 = rbp.tile([128, 2, 512], FP32, tag="rb32", name=f"rb32{lb}")
            for ih in range(2):
                nc.vector.reciprocal_approx_fast(out=rb32[:, ih], in_=ps_rs[ih][:])
            rb_.append(rb32)
        tap("e8", e8_[0][:])
        tap("rb32", rb_[0][:])

        # ---------- phase C x2: attn^T + normalize (fp8 DoubleRow) ----------
        at8_ = []
        for lb in range(BPC):
            # at8: [dp, cp, ib, t, m] so gate lhsT pairs are contiguous
            at8 = at8p.tile([128, NDP, NI, 2, 128], FP8E5, tag="at8", name=f"at8{lb}")
            at8_.append(at8)
            for dc in range(ND):
                cp, t = dc // 2, dc % 2
                ps_a = [psmm.tile([128, 512], FP32, tag="psmm", name=f"psa{lb}_{dc}_{_}") for _ in range(2)]
                for jp in range(NJP):
                    lhsT = pn8_[lb][:, dc, jp]
                    for ih in range(2):
                        nc.tensor.matmul(
                            ps_a[ih],
                            lhsT,
                            e8_[lb][:, jp, ih],
                            start=(jp == 0),
                            stop=(jp == NJP - 1),
                            perf_mode=DR,
                        )
                for ih in range(2):
                    nc.vector.tensor_mul(
                        at8[:, cp, ih * 4 : (ih + 1) * 4, t, :],
                        ps_a[ih].rearrange("p (i m) -> p i m", m=128),
                        rb_[lb][:, ih].rearrange("p (i m) -> p i m", m=128),
                    )
        tap("at8", at8_[0][:])

        # ---------- phase D x2: gates + combine ----------
        for lb in range(BPC):
            for ib in range(NI):
                ps_g = [psmm.tile([128, 512], FP32, tag="psmm", name=f"psg{lb}_{ib}_{_}") for _ in range(3)]
                # all gates fully fp8: P-half e4m3 DR reusing pt8 as lhsT
                # (32x-scaled weights; activations descale via scale=1/32)
                for cp in range(NDP):
                    for g in range(3):
                        nc.tensor.matmul(
                            ps_g[g],
                            pt8_[lb][:, ib, cp],
                            w8f_sb[g][:, 2 * cp : 2 * cp + 2, :],
                            start=(cp == 0),
                            stop=False,
                            perf_mode=DR,
                        )
                for cp in range(NDP):
                    lhsT8 = at8_[lb][:, cp, ib]
                    for g in range(3):
                        nc.tensor.matmul(
                            ps_g[g],
                            lhsT8,
                            w8_sb[g][:, 2 * cp : 2 * cp + 2, :],
                            start=False,
                            stop=(cp == NDP - 1),
                            perf_mode=DR,
                        )
                if with_bias:
                    # bb[1]/bb[2] are pre-scaled by 32 on the host (32x psums)
                    for g in range(3):
                        nc.vector.tensor_add(ps_g[g][:], ps_g[g][:], bb[g][:])
                z16 = gp.tile([128, D], BF16, tag="z16")
                r16 = gp.tile([128, D], BF16, tag="r16")
                f16 = gp.tile([128, D], BF16, tag="f16")
                t16 = tmpp.tile([128, D], BF16, tag="t16")
                rp16 = tmpp.tile([128, D], BF16, tag="rp16")
                o32 = op.tile([128, D], FP32, tag="o32")
                # final block: pipeline the combine in two free-dim halves so
                # the first store overlaps the second half's activations
                # (engines are 128-lane parallel: only free-dim splits shorten
                # op latency)
                halves = (
                    [(0, 256), (256, 512)]
                    if (lb == BPC - 1 and ib == NI - 1)
                    else [(0, 512)]
                )
                for c0, c1 in halves:
                    nc.scalar.activation(
                        z16[:, c0:c1], ps_g[0][:, c0:c1], AF.Tanh, scale=1.0 / 32.0
                    )
                    nc.scalar.activation(
                        f16[:, c0:c1], ps_g[2][:, c0:c1], AF.Sigmoid, scale=1.0 / 32.0
                    )
                    nc.vector.tensor_mul(t16[:, c0:c1], f16[:, c0:c1], z16[:, c0:c1])
                    nc.scalar.activation(
                        r16[:, c0:c1], ps_g[1][:, c0:c1], AF.Sigmoid, scale=1.0 / 32.0
                    )
                    nc.vector.tensor_mul(
                        rp16[:, c0:c1], r16[:, c0:c1], pn16_[lb][:, ib, c0:c1]
                    )
                    nc.vector.tensor_add(o32[:, c0:c1], rp16[:, c0:c1], t16[:, c0:c1])
                    nc.sync.dma_start(
                        out_d[lb, ib * 128 : (ib + 1) * 128, c0:c1], o32[:, c0:c1]
                    )

    nc.compile()
    return nc


def _get_nc(with_bias: bool):
    if with_bias not in _cache:
        _cache[with_bias] = _build(with_bias)
    return _cache[with_bias]


def _prep_in_maps(P, w_atten, w1, w2, w3, b1, b2, b3):
    P = np.ascontiguousarray(np.asarray(P, dtype=np.float32))
    w_atten = np.asarray(w_atten, dtype=np.float32)
    wb = w_atten[D : 2 * D]
    wc = w_atten[2 * D :]
    sj = (P @ wb).reshape(B, NI, 128).astype(np.float32)

    P8 = P.astype(ml_dtypes.float8_e4m3)                       # (B, PL, D)
    Pw8 = (P * (WC_SCALE * wc)[None, None, :]).astype(ml_dtypes.float8_e4m3)
    # pt8: [b, dp, jb, cp, t, m] from P^T[d=(cp,t,dp), j=(jb,m)]
    pt8 = np.ascontiguousarray(
        P8.transpose(0, 2, 1).reshape(B, NDP, 2, 128, NI, 128).transpose(0, 3, 4, 1, 2, 5)
    )
    # pw8: [b, dp, cp, ih, t, ii] from Pw^T[d=(cp,t,dp), i=(ih,ii)]
    pw8 = np.ascontiguousarray(
        Pw8.transpose(0, 2, 1).reshape(B, NDP, 2, 128, 2, 512).transpose(0, 3, 1, 4, 2, 5)
    )
    # pn8: [b, jp_, dc, jp, t, dcol] from P[j=(jp,t,jp_), d=(dc,dcol)]
    pn8 = np.ascontiguousarray(
        P8.reshape(B, NJP, 2, 128, ND, 128).transpose(0, 3, 4, 1, 2, 5)
    )
    P16 = P.astype(ml_dtypes.bfloat16)

    w_all = np.stack(
        [np.asarray(w, dtype=np.float32) for w in (w1, w2, w3)]
    )                                                           # (3, 2D, D)
    w8f = np.ascontiguousarray(
        (32.0 * w_all[:, :D]).reshape(3, ND, 128, D)
    ).astype(ml_dtypes.float8_e4m3)                             # (3, ND, 128, D)
    w8 = np.ascontiguousarray(
        (32.0 * w_all[:, D:]).reshape(3, ND, 128, D).transpose(0, 2, 1, 3)
    ).astype(ml_dtypes.float8_e5m2)                             # (3, 128, ND, D)

    biases = np.stack([np.asarray(b, dtype=np.float32) for b in (b1, b2, b3)])
    biases *= 32.0                                              # all psums are 32x
    with_bias = bool(np.any(biases))
    base = {"w8": w8, "w8f": w8f}
    if with_bias:
        base["b32"] = biases
    in_maps = []
    for c in range(NCORES):
        s = slice(c * BPC, (c + 1) * BPC)
        m = dict(base)
        m["p16"] = P16[s]
        m["pt8"] = pt8[s]
        m["pw8"] = pw8[s]
        m["pn8"] = pn8[s]
        m["sj"] = sj[s]
        in_maps.append(m)
    return in_maps, with_bias


def run(P, w_atten, w1, w2, w3, b1, b2, b3, trace=False):
    in_maps, with_bias = _prep_in_maps(P, w_atten, w1, w2, w3, b1, b2, b3)
    nc = _get_nc(with_bias)
    res = run_bass_kernel_spmd(
        nc, in_maps, core_ids=list(range(NCORES)), trace=trace
    )
    out = np.concatenate([res.results[c]["out"] for c in range(NCORES)], axis=0)
    return out, res


def kernel(P, w_atten, w1, w2, w3, b1, b2, b3):
    out, _ = run(P, w_atten, w1, w2, w3, b1, b2, b3)
    return out
